# revision 2
# baseline (speedup 1.0000x reference)
"""Multi-head attention (B=4, S=2048, D=512, H=8) on 8 TRN2 NeuronCores.

Sharding: the 8192 query rows (4 batches x 2048 seq) are split into 8 shards
of 1024 rows, one per core (core c -> batch c//2, query-half c%2).  Each core
computes full K/V projections for its batch (duplicated across the pair) and
the full output rows for its queries, so no collective is needed — the host
just concatenates the 8 output shards.

Per-core pipeline (all matmuls bf16, accumulation fp32):
  Q^T  [512,1024] = Wq^T.T @ xq^T   (head-pair-chunk layout, d_k on partitions)
  K^T  [512,2048],  V' [2048, 8*(64+1)]  (V plus a ones column per head ->
                                          softmax denominator for free)
  per head: scores^T[k,q] = K^T.T @ Q^T  (k on partitions ->  mask bias is a
            native per-partition bias of the exp activation; no max-pass)
            P^T = exp(0.125*scores^T + maskbias)      (ScalarE, bf16 out)
            pv[65,1024] += V'_h.T @ P^T               (row 64 = denominator)
            x_attn^T = pv[0:64] * bcast(1/pv[64])     (DVE + gpsimd bcast)
  out[q,e] = x_attn^T.T @ Wo^T + bo  -> DRAM

Heads are processed in pairs: even head on PE row-group 0, odd head on
row-group 64 (tile_position auto-derived from base partitions), so the two
K=64 score matmuls run concurrently in the array.
"""
import os
import sys

import numpy as np
import ml_dtypes

try:
    import concourse.bass as bass  # noqa: F401
except ImportError:  # fresh grading dir: fall back to the repo checkout
    for p in ("/root/.axon_site", "/root/.axon_site/_ro/trn_rl_repo",
              "/root/.axon_site/_ro/pypackages", "/opt/trn_rl_repo"):
        if os.path.isdir(p) and p not in sys.path:
            sys.path.insert(0, p)
    import concourse.bass as bass  # noqa: F401

import concourse.mybir as mybir
import concourse.tile as tile
from concourse import bacc
from concourse.bass_utils import run_bass_kernel_spmd

f32 = mybir.dt.float32
bf16 = mybir.dt.bfloat16
BF = ml_dtypes.bfloat16

B, S, D, H, DK = 4, 2048, 512, 8, 64
SQ = S // 2          # queries per core
NKT = S // 128       # 16 key tiles
NDC = D // 128       # 4 contraction chunks
PAIRS = H // 2       # 4 head pairs
EXP = mybir.ActivationFunctionType.Exp
MULT = mybir.AluOpType.mult

_NC_CACHE = None


def build_nc():
    global _NC_CACHE
    if _NC_CACHE is not None:
        return _NC_CACHE
    nc = bacc.Bacc("TRN2", target_bir_lowering=False, debug=False, num_devices=8)

    xq = nc.declare_dram_parameter("xq", [NDC, 128, SQ], bf16, isOutput=False)
    xk = nc.declare_dram_parameter("xk", [NDC, 128, S], bf16, isOutput=False)
    xv = nc.declare_dram_parameter("xv", [NDC, 128, S], bf16, isOutput=False)
    wq = nc.declare_dram_parameter("wq", [NDC, 128, D], bf16, isOutput=False)
    wk = nc.declare_dram_parameter("wk", [NDC, 128, D], bf16, isOutput=False)
    wv = nc.declare_dram_parameter("wv", [NDC, 128, D], bf16, isOutput=False)
    wo = nc.declare_dram_parameter("wo", [NDC, 128, D], bf16, isOutput=False)
    bq = nc.declare_dram_parameter("bq", [1, D], bf16, isOutput=False)
    bk = nc.declare_dram_parameter("bk", [1, D], bf16, isOutput=False)
    bv = nc.declare_dram_parameter("bv", [1, D], bf16, isOutput=False)
    bo = nc.declare_dram_parameter("bo", [1, D], bf16, isOutput=False)
    maskb = nc.declare_dram_parameter("maskb", [128, NKT], f32, isOutput=False)
    out = nc.declare_dram_parameter("out", [SQ, D], f32, isOutput=True)

    with tile.TileContext(nc) as tc:
        with (
            tc.tile_pool(name="const", bufs=1) as cp,
            tc.tile_pool(name="xin", bufs=1) as xin,
            tc.tile_pool(name="pt", bufs=3) as ptp,
            tc.tile_pool(name="den", bufs=3) as dnp,
            tc.tile_pool(name="rbc", bufs=2) as rbp,
            tc.tile_pool(name="outp", bufs=3) as op,
            tc.tile_pool(name="ps_big", bufs=2, space="PSUM") as ps_big,
            tc.tile_pool(name="ps_pv", bufs=2, space="PSUM") as ps_pv,
        ):
            # ---- constants / weights ----
            wq_sb = cp.tile([128, NDC, D], bf16, tag="wq")
            wk_sb = cp.tile([128, NDC, D], bf16, tag="wk")
            wv_sb = cp.tile([128, NDC, D], bf16, tag="wv")
            wo_sb = cp.tile([128, NDC, D], bf16, tag="wo")
            nc.sync.dma_start(wq_sb[:], wq[:].rearrange("c p n -> p c n"))
            nc.sync.dma_start(wk_sb[:], wk[:].rearrange("c p n -> p c n"))
            nc.sync.dma_start(wv_sb[:], wv[:].rearrange("c p n -> p c n"))
            nc.sync.dma_start(wo_sb[:], wo[:].rearrange("c p n -> p c n"))
            bq_sb = cp.tile([1, D], bf16, tag="bq")
            bk_sb = cp.tile([1, D], bf16, tag="bk")
            bv_sb = cp.tile([1, D], bf16, tag="bv")
            bo_sb = cp.tile([1, D], bf16, tag="bo")
            nc.sync.dma_start(bq_sb[:], bq[:])
            nc.sync.dma_start(bk_sb[:], bk[:])
            nc.sync.dma_start(bv_sb[:], bv[:])
            nc.sync.dma_start(bo_sb[:], bo[:])
            ones_sb = cp.tile([1, D], bf16, tag="ones")
            nc.vector.memset(ones_sb[:], 1.0)
            maskb_sb = cp.tile([128, NKT], f32, tag="maskb")
            nc.sync.dma_start(maskb_sb[:], maskb[:])

            # ---- persistent activations ----
            QT_sb = cp.tile([128, PAIRS, SQ], bf16, tag="QT")
            KT_sb = cp.tile([128, PAIRS, S], bf16, tag="KT")
            VP_sb = cp.tile([128, NKT, H * 65], bf16, tag="VP")
            XA_sb = cp.tile([128, PAIRS, SQ], bf16, tag="XA")
            # ones column per head inside V'
            vp_ones = VP_sb[:].rearrange("p k (h c) -> p k h c", c=65)[:, :, :, 64:65]
            nc.vector.memset(vp_ones, 1.0)

            xq_c = [xin.tile([128, SQ], bf16, tag=f"xq{dc}", name=f"xq{dc}") for dc in range(NDC)]
            xk_c = [xin.tile([128, S], bf16, tag=f"xk{dc}", name=f"xk{dc}") for dc in range(NDC)]
            xv_c = [xin.tile([128, S], bf16, tag=f"xv{dc}", name=f"xv{dc}") for dc in range(NDC)]
            for dc in range(NDC):
                nc.sync.dma_start(xq_c[dc][:], xq[dc])
                nc.sync.dma_start(xk_c[dc][:], xk[dc])
                nc.sync.dma_start(xv_c[dc][:], xv[dc])

            # ---- emission helpers ----
            def proj_QT(c):
                for qch in range(SQ // 512):
                    ps = ps_big.tile([128, 1024], f32, tag="big")
                    for dc in range(NDC):
                        nc.tensor.matmul(
                            ps[:, 0:512],
                            wq_sb[:, dc, c * 128:(c + 1) * 128],
                            xq_c[dc][:, qch * 512:(qch + 1) * 512],
                            start=(dc == 0), stop=False,
                        )
                    nc.tensor.matmul(
                        ps[:, 0:512],
                        bq_sb[0:1, c * 128:(c + 1) * 128],
                        ones_sb[0:1, 0:512],
                        start=False, stop=True,
                    )
                    nc.vector.tensor_copy(
                        QT_sb[:, c, qch * 512:(qch + 1) * 512], ps[:, 0:512]
                    )

            def proj_KT(c):
                for tch in range(S // 512):
                    ps = ps_big.tile([128, 1024], f32, tag="big")
                    for dc in range(NDC):
                        nc.tensor.matmul(
                            ps[:, 0:512],
                            wk_sb[:, dc, c * 128:(c + 1) * 128],
                            xk_c[dc][:, tch * 512:(tch + 1) * 512],
                            start=(dc == 0), stop=False,
                        )
                    nc.tensor.matmul(
                        ps[:, 0:512],
                        bk_sb[0:1, c * 128:(c + 1) * 128],
                        ones_sb[0:1, 0:512],
                        start=False, stop=True,
                    )
                    nc.vector.tensor_copy(
                        KT_sb[:, c, tch * 512:(tch + 1) * 512], ps[:, 0:512]
                    )

            def proj_V(kt):
                ps = ps_big.tile([128, 1024], f32, tag="big")
                for dc in range(NDC):
                    nc.tensor.matmul(
                        ps[:, 0:512],
                        xv_c[dc][:, kt * 128:(kt + 1) * 128],
                        wv_sb[:, dc, :],
                        start=(dc == 0), stop=False,
                    )
                nc.tensor.matmul(
                    ps[:, 0:512],
                    ones_sb[0:1, 0:128],
                    bv_sb[0:1, :],
                    start=False, stop=True,
                )
                nc.vector.tensor_copy(
                    VP_sb[:, kt].rearrange("p (h c) -> p h c", c=65)[:, :, 0:64],
                    ps[:, 0:512].rearrange("p (h c) -> p h c", c=64),
                )

            def attention_pair(c):
                pv = [ps_pv.tile([128, 1024], f32, tag="pv", name=f"pv{c}_{i}") for i in range(2)]
                for kt in range(NKT):
                    if c == 0:
                        proj_V(kt)
                    if kt == 10 and c < PAIRS - 1:
                        proj_KT(c + 1)
                    pt = ptp.tile([128, 2048], bf16, tag="pt")
                    for half in range(2):
                        lo, hi = half * 64, (half + 1) * 64
                        sc = ps_big.tile([128, 1024], f32, tag="big")
                        for qch in range(2):
                            nc.tensor.matmul(
                                sc[:, qch * 512:(qch + 1) * 512],
                                KT_sb[lo:hi, c, kt * 128:(kt + 1) * 128],
                                QT_sb[lo:hi, c, qch * 512:(qch + 1) * 512],
                                start=True, stop=True,
                            )
                        nc.scalar.activation(
                            pt[:, half * 1024:(half + 1) * 1024],
                            sc[:],
                            EXP,
                            bias=maskb_sb[:, kt:kt + 1],
                            scale=0.125,
                        )
                        h = 2 * c + half
                        for qch in range(2):
                            nc.tensor.matmul(
                                pv[half][0:65, qch * 512:(qch + 1) * 512],
                                VP_sb[:, kt, h * 65:(h + 1) * 65],
                                pt[:, half * 1024 + qch * 512:
                                   half * 1024 + (qch + 1) * 512],
                                start=(kt == 0), stop=(kt == NKT - 1),
                            )
                for half in range(2):
                    den = dnp.tile([1, SQ], f32, tag="den")
                    nc.vector.tensor_copy(den[:], pv[half][64:65, 0:SQ])
                    rec = dnp.tile([1, SQ], f32, tag="rec")
                    scr = dnp.tile([1, SQ], f32, tag="scr")
                    nc.vector.reciprocal_approx_accurate(
                        out=rec[:], in_=den[:], scratch=scr[:]
                    )
                    rbc = rbp.tile([64, SQ], f32, tag="rbc")
                    nc.gpsimd.partition_broadcast(rbc[:], rec[:])
                    nc.vector.tensor_tensor(
                        XA_sb[half * 64:(half + 1) * 64, c, :],
                        pv[half][0:64, 0:SQ],
                        rbc[:],
                        MULT,
                    )

            def wo_out(qt):
                ps = ps_pv.tile([128, 1024], f32, tag="pv")
                for j in range(NDC):
                    nc.tensor.matmul(
                        ps[:, 0:512],
                        XA_sb[:, j, qt * 128:(qt + 1) * 128],
                        wo_sb[:, j, :],
                        start=(j == 0), stop=False,
                    )
                nc.tensor.matmul(
                    ps[:, 0:512],
                    ones_sb[0:1, 0:128],
                    bo_sb[0:1, :],
                    start=False, stop=True,
                )
                ob = op.tile([128, D], f32, tag="ob")
                nc.vector.tensor_copy(ob[:], ps[:, 0:512])
                nc.sync.dma_start(out[qt * 128:(qt + 1) * 128, :], ob[:])

            # ---- schedule ----
            for c in range(PAIRS):
                proj_QT(c)
            proj_KT(0)
            for c in range(PAIRS):
                attention_pair(c)
            for qt in range(SQ // 128):
                wo_out(qt)

    nc.finalize()
    _NC_CACHE = nc
    return nc


def make_in_maps(query, key, value, mask, Wq, bq, Wk, bk, Wv, bv, Wo, bo):
    query = np.asarray(query, np.float32)
    key = np.asarray(key, np.float32)
    value = np.asarray(value, np.float32)
    mask = np.asarray(mask)

    def wprep(W):
        return np.ascontiguousarray(
            np.asarray(W, np.float32).T.reshape(NDC, 128, D)
        ).astype(BF)

    wq_a, wk_a, wv_a, wo_a = wprep(Wq), wprep(Wk), wprep(Wv), wprep(Wo)
    bq_a = np.asarray(bq, np.float32).reshape(1, D).astype(BF)
    bk_a = np.asarray(bk, np.float32).reshape(1, D).astype(BF)
    bv_a = np.asarray(bv, np.float32).reshape(1, D).astype(BF)
    bo_a = np.asarray(bo, np.float32).reshape(1, D).astype(BF)

    kT = key.transpose(0, 2, 1)    # [B, D, S]
    vT = value.transpose(0, 2, 1)
    qT = query.transpose(0, 2, 1)

    in_maps = []
    for core in range(8):
        b, qh = core // 2, core % 2
        xq_a = np.ascontiguousarray(
            qT[b][:, qh * SQ:(qh + 1) * SQ]).reshape(NDC, 128, SQ).astype(BF)
        xk_a = np.ascontiguousarray(kT[b]).reshape(NDC, 128, S).astype(BF)
        xv_a = np.ascontiguousarray(vT[b]).reshape(NDC, 128, S).astype(BF)
        mb = np.where(mask[b, 0] == 0, np.float32(-1e9), np.float32(0.0))
        mb = np.ascontiguousarray(mb.reshape(NKT, 128).T).astype(np.float32)
        in_maps.append({
            "xq": xq_a, "xk": xk_a, "xv": xv_a,
            "wq": wq_a, "wk": wk_a, "wv": wv_a, "wo": wo_a,
            "bq": bq_a, "bk": bk_a, "bv": bv_a, "bo": bo_a,
            "maskb": mb,
        })
    return in_maps


def assemble_output(results):
    full = np.empty((B, S, D), np.float32)
    for core in range(8):
        b, qh = core // 2, core % 2
        full[b, qh * SQ:(qh + 1) * SQ, :] = results[core]["out"]
    return full


def kernel(**inputs):
    nc = build_nc()
    in_maps = make_in_maps(**inputs)
    res = run_bass_kernel_spmd(nc, in_maps, list(range(8))).results
    return assemble_output(res)
